# revision 43
# baseline (speedup 1.0000x reference)
"""Trainium2 Bass kernel for a pre-norm transformer block (B=4, N=1024, C=1024, H=16).

Sharding: 8 cores, each handles 512 tokens (one half-sequence of one batch).
K/V are recomputed per-core for the full 1024-token sequence (no collectives).
Own tokens are rotated to rows 0:511 of the per-core x input so all cores run
one SPMD program (attention is permutation-invariant over keys).

Per-core dataflow (activations channel-major where matmuls contract over channels):
  LN1 (fp32 stats on DVE, normalize on ACT; affine folded into weights on host)
  PE-transpose h -> hT [c, tok]
  qT = WqkT' @ hT_own ; kT = full ; v = hT.T @ WvT'  (tokens-major, +ones col -> v65)
  per head-pair: sT(2 heads) -> one [128,1024] psum -> one Exp ACT (scale=1/8) -> ET bf16
            o65 = v65_h.T @ ET_h (psum accum over key chunks; row 64 = softmax denom)
            oT_h = o65[0:64] * (1/o65[64]) partition-broadcast
  proj: x2 = x_own + proj_b + oT.T @ projT  (LN2 bn_stats fused into eviction)
  LN2 -> h2 -> transpose -> fc1T (w1 streamed) -> Gelu(+bias) -> g
  fc2 (w2 streamed, hj-blocked, accumulated into x2 in SBUF) -> out
"""

import contextlib

import numpy as np
import ml_dtypes

import concourse.bass as bass
import concourse.mybir as mybir
import concourse.tile as tile
from concourse import bacc
from concourse.bass_utils import run_bass_kernel_spmd
from concourse.masks import make_identity

BF16 = mybir.dt.bfloat16
F32 = mybir.dt.float32
AF = mybir.ActivationFunctionType
ALU = mybir.AluOpType

B, N, C = 4, 1024, 1024
H, D = 16, 64
HID = 4 * C
NOWN = 512          # tokens owned per core
EPS = 1e-5
P = 128
CC = C // P         # 8 channel chunks
NJ = NOWN // P      # 4 own-token chunks
MJ = N // P         # 8 key-token chunks
HJ = HID // P       # 32 hidden chunks
FT = 512            # matmul free tile


def build_program():
    nc = bacc.Bacc("TRN2", target_bir_lowering=False, debug=False, num_devices=8)

    xb = nc.dram_tensor("xb", [N, C], F32, kind="ExternalInput").ap()
    wqk = nc.dram_tensor("wqk", [P, CC * 2 * C], BF16, kind="ExternalInput").ap()
    wv = nc.dram_tensor("wv", [P, CC * C], BF16, kind="ExternalInput").ap()
    wp = nc.dram_tensor("wp", [P, CC * C], BF16, kind="ExternalInput").ap()
    w1 = nc.dram_tensor("w1", [P, 4 * CC * 1024], BF16, kind="ExternalInput").ap()
    w2 = nc.dram_tensor("w2", [P, 4 * 8 * C], BF16, kind="ExternalInput").ap()
    qkb = nc.dram_tensor("qkb", [P, 16], F32, kind="ExternalInput").ap()
    vb = nc.dram_tensor("vb", [C], F32, kind="ExternalInput").ap()
    pb = nc.dram_tensor("pb", [C], F32, kind="ExternalInput").ap()
    f1b = nc.dram_tensor("f1b", [P, HJ], F32, kind="ExternalInput").ap()
    f2b = nc.dram_tensor("f2b", [C], F32, kind="ExternalInput").ap()
    idin = nc.dram_tensor("idin", [P, P], BF16, kind="ExternalInput").ap()
    out = nc.dram_tensor("out", [NOWN, C], F32, kind="ExternalOutput").ap()

    xv = xb.rearrange("(j p) c -> p j c", p=P)          # [128, 8, 1024]
    wqkv = wqk.rearrange("p (h c o) -> p h c o", h=2, c=CC)  # [128, 2, 8, 1024]
    ov = out.rearrange("(j p) c -> p j c", p=P)         # [128, 4, 1024]
    w1v = w1.rearrange("p (q c o) -> p q c o", q=4, c=CC)   # [128, 4, 8, 1024]
    w2v = w2.rearrange("p (q h o) -> p q h o", q=4, h=8)    # [128, 4, 8, 1024]

    def bcast_row(v):
        # DRAM [C] -> broadcast across partitions [P, C]
        return bass.AP(tensor=v.tensor, offset=v.offset, ap=[[0, P], v.ap[0]])

    with tile.TileContext(nc) as tc:
        ctx = contextlib.ExitStack()
        with ctx:
            # ---- long-lived pools (pop at the very end) ----
            const = ctx.enter_context(tc.tile_pool(name="const", bufs=1, side="left"))
            psum = ctx.enter_context(tc.tile_pool(name="psum", bufs=2, space="PSUM"))
            pstat = ctx.enter_context(tc.tile_pool(name="pstat", bufs=4, side="left"))
            pet = ctx.enter_context(tc.tile_pool(name="pet", bufs=3, side="left"))
            pr = ctx.enter_context(tc.tile_pool(name="pr", bufs=4, side="left"))

            # ---- phase-scoped pools, LIFO per side (left) ----
            satt = contextlib.ExitStack()   # patt, pxpb   (pops after G)
            shT = contextlib.ExitStack()    # phT          (pops after E)
            sh = contextlib.ExitStack()     # ph           (pops after C)
            sx = contextlib.ExitStack()     # px           (pops after B)
            patt = satt.enter_context(tc.tile_pool(name="patt", bufs=1, side="left"))
            pxpb = satt.enter_context(tc.tile_pool(name="pxpb", bufs=1, side="left"))
            phT = shT.enter_context(tc.tile_pool(name="phT", bufs=1, side="left"))
            ph = sh.enter_context(tc.tile_pool(name="ph", bufs=2, side="left"))
            px = sx.enter_context(tc.tile_pool(name="px", bufs=1, side="left"))

            swqv = contextlib.ExitStack()
            pwqv = swqv.enter_context(tc.tile_pool(name="pwqv", bufs=1,
                                                   side="right"))
            wqk_s = pwqv.tile([P, 2, CC, 1024], BF16)
            wv_s = pwqv.tile([P, CC, C], BF16)
            x_s = px.tile([P, MJ, C], F32)
            nc.sync.dma_start(x_s[:, 0, :], xv[:, 0, :])
            ident = const.tile([P, P], BF16)
            nc.sync.dma_start(ident, idin)
            for j in range(1, NJ):
                nc.sync.dma_start(x_s[:, j, :], xv[:, j, :])
            nc.sync.dma_start(wqk_s[:, 0], wqkv[:, 0])     # q-half weights
            eps_t = const.tile([P, 1], F32)
            nc.vector.memset(eps_t, EPS)
            qkb_s = const.tile([P, 16], F32)
            nc.sync.dma_start(qkb_s, qkb)
            for j in range(NJ, MJ):
                nc.sync.dma_start(x_s[:, j, :], xv[:, j, :])
            nc.sync.dma_start(wqk_s[:, 1], wqkv[:, 1])     # k-half weights
            nc.sync.dma_start(wv_s, wv.rearrange("p (c o) -> p c o", c=CC))
            vb_s = const.tile([P, C], F32)
            nc.sync.dma_start(vb_s, bcast_row(vb))
            f1b_s = const.tile([P, HJ], F32)
            nc.sync.dma_start(f1b_s, f1b)
            pb_s = const.tile([P, C], F32)
            nc.sync.dma_start(pb_s, bcast_row(pb))
            f2b_s = const.tile([P, C], F32)
            nc.sync.dma_start(f2b_s, bcast_row(f2b))

            _rot = [0]

            def mm_psum():
                # rotate across ps/po/sc slots (sc+po idle outside attention)
                _rot[0] += 1
                tag = ("ps", "po", "sc")[_rot[0] % 3]
                if tag == "sc":
                    t = psum.tile([P, 2 * FT], F32, tag="sc", bufs=2,
                                  name=f"mm_{_rot[0]}")
                    return t[:, 0:FT]
                return psum.tile([P, FT], F32, tag=tag, bufs=2,
                                 name=f"mm_{_rot[0]}")

            def ln_stats(src, st):
                # src [P, C] f32; st [P, 2, 6] bn_stats output (two 512-halves)
                sv = src.rearrange("p (s f) -> p s f", s=2)
                nc.vector.bn_stats(out=st[:, 0, :], in_=sv[:, 0, :])
                nc.vector.bn_stats(out=st[:, 1, :], in_=sv[:, 1, :])

            def ln_normalize(src, dst, st):
                # finish stats -> dst = (src - mean) * rstd on ACT (bf16 out)
                mv = pstat.tile([P, 2], F32, tag="mv")
                nc.vector.bn_aggr(out=mv, in_=st)
                rs = pstat.tile([P, 1], F32, tag="rs")
                nc.scalar.activation(out=rs, in_=mv[:, 1:2],
                                     func=AF.Abs_reciprocal_sqrt,
                                     bias=eps_t, scale=1.0)
                nb = pstat.tile([P, 1], F32, tag="nb")
                nc.vector.tensor_scalar(out=nb, in0=mv[:, 0:1], scalar1=rs,
                                        scalar2=-1.0, op0=ALU.mult,
                                        op1=ALU.mult)
                nc.scalar.activation(out=dst, in_=src, func=AF.Identity,
                                     bias=nb, scale=rs)

            # ------ Phase A/B/C1: LN1 + transpose per own chunk, then q ------
            hT_s = phT.tile([P, CC, N], BF16)
            q_s = patt.tile([P, CC, NOWN], BF16, tag="q")
            kT_s = patt.tile([P, CC, N], BF16, tag="k")
            v65_s = patt.tile([P, MJ, H, 65], BF16, tag="v")
            oT_s = patt.tile([P, CC, NOWN], BF16, tag="o")
            nc.vector.memset(v65_s[:, :, :, 64:65], 1.0)
            xpb_s = pxpb.tile([P, NJ, C], F32)

            def ln_transpose_chunk(j):
                st = pstat.tile([P, 2, 6], F32, tag="st")
                ln_stats(x_s[:, j, :], st)
                h = ph.tile([P, C], BF16, tag="h")
                ln_normalize(x_s[:, j, :], h, st)
                for i in range(CC):
                    pt = psum.tile([P, P], BF16, tag="ps" if i % 2 else "po",
                                   bufs=2)
                    nc.tensor.transpose(pt, h[:, i * P:(i + 1) * P], ident)
                    nc.vector.tensor_copy(out=hT_s[:, i, j * P:(j + 1) * P],
                                          in_=pt)

            for j in range(NJ):
                ln_transpose_chunk(j)
            for oi in range(CC):            # q chunks (own tokens only)
                pm = mm_psum()
                for cc in range(CC):
                    nc.tensor.matmul(
                        pm, wqk_s[:, 0, cc, oi * P:(oi + 1) * P],
                        hT_s[:, cc, 0:FT],
                        start=(cc == 0), stop=(cc == CC - 1))
                nc.vector.tensor_scalar(out=q_s[:, oi, :], in0=pm,
                                        scalar1=qkb_s[:, oi:oi + 1],
                                        scalar2=None, op0=ALU.add)

            # ---- Phase C2: LN + transpose remaining chunks, then kT / v ----
            for j in range(NJ, MJ):
                ln_transpose_chunk(j)
            for j in range(NJ):
                nc.vector.tensor_add(out=xpb_s[:, j, :], in0=x_s[:, j, :],
                                     in1=pb_s)
            sx.close()
            sh.close()
            for oi in range(CC):            # k chunks (all tokens)
                for t in range(2):
                    pm = mm_psum()
                    for cc in range(CC):
                        nc.tensor.matmul(
                            pm, wqk_s[:, 1, cc, oi * P:(oi + 1) * P],
                            hT_s[:, cc, t * FT:(t + 1) * FT],
                            start=(cc == 0), stop=(cc == CC - 1))
                    nc.vector.tensor_scalar(
                        out=kT_s[:, oi, t * FT:(t + 1) * FT], in0=pm,
                        scalar1=qkb_s[:, CC + oi:CC + oi + 1],
                        scalar2=None, op0=ALU.add)
            # ------------ Phase E: v (tokens-major) + ones ------------
            for m in range(MJ):
                for t in range(2):
                    pm = mm_psum()
                    for cc in range(CC):
                        nc.tensor.matmul(
                            pm, hT_s[:, cc, m * P:(m + 1) * P],
                            wv_s[:, cc, t * FT:(t + 1) * FT],
                            start=(cc == 0), stop=(cc == CC - 1))
                    nc.vector.tensor_tensor(
                        out=v65_s[:, m, t * 8:(t + 1) * 8, 0:64],
                        in0=pm.rearrange("p (h d) -> p h d", d=D),
                        in1=vb_s[:, t * FT:(t + 1) * FT].rearrange(
                            "p (h d) -> p h d", d=D),
                        op=ALU.add)
            swqv.close()
            shT.close()

            # ---- Phase F: attention (head pairs share one Exp), proj folded --
            # proj partial of pair hp-1 is emitted inside pair hp's region so
            # the PE has ready work while Exp (ACT) paces the softmax.
            px2 = ctx.enter_context(tc.tile_pool(name="px2", bufs=1, side="right"))
            pw1s = ctx.enter_context(tc.tile_pool(name="pw1s", bufs=2, side="right"))
            ph2T = ctx.enter_context(tc.tile_pool(name="ph2T", bufs=1, side="right"))
            ph2 = ctx.enter_context(tc.tile_pool(name="ph2", bufs=4, side="right"))
            x2_s = px2.tile([P, NJ, C], F32)
            h2T_s = ph2T.tile([P, CC, NOWN], BF16)
            h2_l = [ph2.tile([P, C], BF16, tag="h2", name=f"h2_{nj}")
                    for nj in range(NJ)]
            st2 = [pstat.tile([P, 2, 6], F32, tag=f"st2_{nj}", name=f"st2_{nj}")
                   for nj in range(NJ)]

            def proj_partial(hp, njs=range(NJ)):
                # x2 += oT[:, hp, :].T @ wp[:, hp, :] (pair hp = channel chunk hp)
                for nj in njs:
                    for t in range(2):
                        pm = psum.tile([P, FT], F32, tag="ps")
                        nc.tensor.matmul(
                            pm, oT_s[:, hp, nj * P:(nj + 1) * P],
                            wp_s[:, hp, t * FT:(t + 1) * FT],
                            start=True, stop=True)
                        nc.vector.tensor_tensor(
                            out=x2_s[:, nj, t * FT:(t + 1) * FT], in0=pm,
                            in1=(xpb_s if hp == 0 else x2_s)[
                                :, nj, t * FT:(t + 1) * FT],
                            op=ALU.add)
                        if hp == H // 2 - 1:
                            nc.vector.bn_stats(
                                out=st2[nj][:, t, :],
                                in_=x2_s[:, nj, t * FT:(t + 1) * FT])
                            if t == 1:
                                # LN2 chain for this chunk (DVE/ACT only)
                                ln_normalize(x2_s[:, nj, :], h2_l[nj], st2[nj])
                                nc.vector.tensor_add(out=x2_s[:, nj, :],
                                                     in0=x2_s[:, nj, :],
                                                     in1=f2b_s)

            with tc.tile_pool(name="pwp", bufs=1, side="right") as pwp:
                wp_s = pwp.tile([P, CC, C], BF16)
                nc.sync.dma_start(wp_s, wp.rearrange("p (c o) -> p c o", c=CC))
                for hp in range(H // 2):
                    oc = hp
                    sc = psum.tile([P, 2 * FT], F32, tag="sc", bufs=2)
                    po0 = psum.tile([P, FT], F32, tag="po", bufs=2)
                    po1 = psum.tile([P, FT], F32, tag="po", bufs=2)
                    et = pet.tile([P, 2 * FT], BF16, tag="et")
                    for m in range(MJ):
                        for half in range(2):
                            lo = half * D
                            nc.tensor.matmul(
                                sc[:, half * FT:(half + 1) * FT],
                                kT_s[lo:lo + D, oc, m * P:(m + 1) * P],
                                q_s[lo:lo + D, oc, :],
                                start=True, stop=True,
                                tile_position=(lo, 0))
                        nc.scalar.activation(out=et, in_=sc, func=AF.Exp,
                                             scale=0.125)
                        nc.tensor.matmul(po0[:65, :], v65_s[:, m, 2 * hp, :],
                                         et[:, 0:FT],
                                         start=(m == 0), stop=(m == MJ - 1))
                        nc.tensor.matmul(po1[:65, :], v65_s[:, m, 2 * hp + 1, :],
                                         et[:, FT:2 * FT],
                                         start=(m == 0), stop=(m == MJ - 1))
                        if m == 1 and hp > 0:
                            proj_partial(hp - 1, (0, 1))
                        if m == 4 and hp > 0:
                            proj_partial(hp - 1, (2, 3))
                        if m != MJ - 1:
                            sc = psum.tile([P, 2 * FT], F32, tag="sc", bufs=2)
                            et = pet.tile([P, 2 * FT], BF16, tag="et")
                    for half, po in ((0, po0), (1, po1)):
                        lo = half * D
                        # copy out fast to release the psum bank, then finish
                        # the normalization chain off the critical path
                        o65 = pr.tile([65, FT], F32, tag="o65", bufs=3)
                        nc.vector.tensor_copy(out=o65, in_=po[:65, :])
                        rinv = pr.tile([1, FT], F32, tag="rinv")
                        nc.vector.reciprocal(out=rinv, in_=o65[64:65, :])
                        rb = pr.tile([D, FT], F32, tag="rb")
                        nc.gpsimd.partition_broadcast(rb, rinv)
                        nc.vector.tensor_tensor(out=oT_s[lo:lo + D, oc, :],
                                                in0=o65[0:64, :], in1=rb,
                                                op=ALU.mult)
                proj_partial(H // 2 - 1)
                for i in range(CC):
                    for nj in range(NJ):
                        pt = psum.tile([P, P], BF16, tag="ps" if nj % 2 else "po",
                                       bufs=2)
                        nc.tensor.transpose(
                            pt, h2_l[nj][:, i * P:(i + 1) * P], ident)
                        nc.vector.tensor_copy(
                            out=h2T_s[:, i, nj * P:(nj + 1) * P], in_=pt)
            satt.close()

            # ---------------- Phase H handled inside proj_partial ------------
            pg = ctx.enter_context(tc.tile_pool(name="pg", bufs=1, side="right"))
            pw2s = ctx.enter_context(tc.tile_pool(name="pw2s", bufs=2, side="right"))

            # ---------------- Phase I: fc1 + gelu (w1 streamed) ---------------
            g_s = pg.tile([P, HJ, NOWN], BF16)
            for hjq in range(4):
                w1c = pw1s.tile([P, CC, 1024], BF16, tag="w1c")
                nc.sync.dma_start(w1c, w1v[:, hjq])
                for hjj in range(8):
                    hj = hjq * 8 + hjj
                    pm = mm_psum()
                    for cc in range(CC):
                        nc.tensor.matmul(
                            pm, w1c[:, cc, hjj * P:(hjj + 1) * P],
                            h2T_s[:, cc, :],
                            start=(cc == 0), stop=(cc == CC - 1))
                    nc.scalar.activation(out=g_s[:, hj, :], in_=pm, func=AF.Gelu,
                                         bias=f1b_s[:, hj:hj + 1], scale=1.0)

            # ------- Phase J: fc2 (w2 streamed, hj-blocked into x2) -> out ----
            for hjq in range(4):
                w2c = pw2s.tile([P, 8, C], BF16, tag="w2c")
                nc.sync.dma_start(w2c, w2v[:, hjq])
                for nj in range(NJ):
                    for t in range(2):
                        pm = mm_psum()
                        for hj8 in range(8):
                            nc.tensor.matmul(
                                pm, g_s[:, hjq * 8 + hj8, nj * P:(nj + 1) * P],
                                w2c[:, hj8, t * FT:(t + 1) * FT],
                                start=(hj8 == 0), stop=(hj8 == 7))
                        nc.vector.tensor_add(
                            out=x2_s[:, nj, t * FT:(t + 1) * FT], in0=pm,
                            in1=x2_s[:, nj, t * FT:(t + 1) * FT])
                        if hjq == 3:
                            nc.sync.dma_start(
                                out=ov[:, nj, t * FT:(t + 1) * FT],
                                in_=x2_s[:, nj, t * FT:(t + 1) * FT])

    nc.compile()
    return nc


_NC = None


def _get_program():
    global _NC
    if _NC is None:
        _NC = build_program()
    return _NC


def _prep_in_maps(x, ln1_w, ln1_b, qkv_w, proj_w, proj_b, ln2_w, ln2_b,
                  fc1_w, fc1_b, fc2_w, fc2_b):
    bf = ml_dtypes.bfloat16
    f32 = np.float32
    ln1_w, ln1_b = np.asarray(ln1_w, f32), np.asarray(ln1_b, f32)
    ln2_w, ln2_b = np.asarray(ln2_w, f32), np.asarray(ln2_b, f32)
    qkv_w = np.asarray(qkv_w, f32)
    proj_w = np.asarray(proj_w, f32)
    fc1_w = np.asarray(fc1_w, f32)
    fc2_w = np.asarray(fc2_w, f32)

    def chunk_cmajor(wT, cols):
        # wT [C_in, cols] -> [128, (C_in/128) * cols] with c = cc*128 + p
        return np.ascontiguousarray(
            wT.reshape(-1, P, cols).transpose(1, 0, 2).reshape(P, -1)).astype(bf)

    wqk_w = (qkv_w[:2 * C] * ln1_w[None, :]).T          # [C, 2C]
    wv_w = (qkv_w[2 * C:] * ln1_w[None, :]).T           # [C, C]
    w1_w = (fc1_w * ln2_w[None, :]).T                   # [C, HID]
    # w1: [128, 4(hjq), 8(cc), 1024] ; c = cc*128+p, h' = hjq*1024 + o
    w1_host = np.ascontiguousarray(
        w1_w.reshape(CC, P, 4, 1024).transpose(1, 2, 0, 3).reshape(P, -1)
    ).astype(bf)
    # w2: [128, 4(hjq), 8(hj8), 1024(c)] ; h' = (hjq*8+hj8)*128 + p
    w2_host = np.ascontiguousarray(
        fc2_w.T.reshape(4, 8, P, C).transpose(2, 0, 1, 3).reshape(P, -1)
    ).astype(bf)
    shared = {
        "wqk": np.concatenate(
            [chunk_cmajor(np.ascontiguousarray(wqk_w[:, :C]), C),
             chunk_cmajor(np.ascontiguousarray(wqk_w[:, C:]), C)],
            axis=1),
        "wv": chunk_cmajor(wv_w, C),
        "wp": chunk_cmajor(np.ascontiguousarray(proj_w.T), C),
        "w1": w1_host,
        "w2": w2_host,
        "qkb": np.ascontiguousarray(
            (qkv_w[:2 * C] @ ln1_b).reshape(16, P).T).astype(f32),
        "vb": np.ascontiguousarray(qkv_w[2 * C:] @ ln1_b).astype(f32),
        "pb": np.ascontiguousarray(np.asarray(proj_b, f32)),
        "f1b": np.ascontiguousarray(
            (np.asarray(fc1_b, f32) + fc1_w @ ln2_b).reshape(HJ, P).T).astype(f32),
        "f2b": np.ascontiguousarray(np.asarray(fc2_b, f32)),
        "idin": np.eye(P, dtype=bf),
    }
    x = np.asarray(x, f32)
    in_maps = []
    for core in range(8):
        b, hf = core // 2, core % 2
        own = x[b, hf * NOWN:(hf + 1) * NOWN]
        other = x[b, (1 - hf) * NOWN:(2 - hf) * NOWN]
        xb_c = np.ascontiguousarray(np.concatenate([own, other], axis=0))
        in_maps.append({**shared, "xb": xb_c})
    return in_maps


def kernel(**inputs) -> np.ndarray:
    nc = _get_program()
    in_maps = _prep_in_maps(**inputs)
    res = run_bass_kernel_spmd(nc, in_maps, core_ids=list(range(8)))
    out = np.empty((B, N, C), np.float32)
    for core in range(8):
        b, hf = core // 2, core % 2
        out[b, hf * NOWN:(hf + 1) * NOWN] = res.results[core]["out"]
    return out
